# revision 8
# baseline (speedup 1.0000x reference)
"""Trainium2 Bass kernel for a 2-layer DGCN (graph conv) on 8 NeuronCores.

Reference computation (fp32):
    h1  = relu(IFadj @ (x @ W1) + b1)         # [N, NHID]
    out = BN(adj @ (h1 @ W2) + b2)            # [N, OUTD], BN in eval mode

Distribution: rows of x / IFadj / adj are sharded across 8 cores
(row-parallel graph partitioning).  Matmul phases 1, 2, 4 run in fp8
e4m3 DoubleRow mode (two 128-deep k-tiles per instruction, 2x the bf16
rate); phase 3 (h1 @ W2) stays bf16 because W2-quantization error is
coherent across nodes (h1 has a positive mean) and would be amplified
by adj's mean in phase 4.

fp8 accuracy: the dominant quantization error is the column-sum of the
S = x@W1 residual, amplified by the means of BOTH IFadj and adj.  That
error is rank-1 -- rowsum(IFadj8)/N x colsum(S_true - S8) -- and is
cancelled by ONE extra K=1 matmul accumulated into each phase-2 PSUM
group; the u vector is computed on the host (it also absorbs phase-1's
own fp8 error).  W1 is pre-scaled by 8 (and S by 1/8 on the PSUM
evict) to keep everything in e4m3's normal range.

Latency structure (all DMAs share one in-order DGE queue, collectives
have ~16 us fixed latency, and the PE clock drops to 1.2 GHz after any
idle gap):
  - a dummy AllGather on b1p is issued first so the collective
    bootstrap/skew barrier runs concurrently with the DMA preamble;
  - dummy matmuls on a zeroed tile keep the PE busy (and its clock
    ramping) while the phase-1 operands stream in;
  - gather-dependent staging DMAs are issued strictly after the
    independent loads they would otherwise block in the queue;
  - phase-4's adj stream is prefetched during phase 2 so phase 4 is
    not DMA-bound;
  - each core redundantly computes the S rows of 4 (chunk,core) groups
    (the "tail") and phase 2 consumes those first, covering the S
    allgather latency.
"""

import numpy as np
import ml_dtypes

NCORES = 8
N = 8192
NFEAT = 1024
NHID = 512
OUTD = 256
ROWS = N // NCORES  # 1024
P = 128
BN_EPS = 1e-5

CB = NFEAT // P   # 8  c-blocks (x feature contraction)
CP = CB // 2      # 4  c-pairs (DoubleRow)
IB = ROWS // P    # 8  i-blocks (local rows)
JB = NHID // P    # 4  j-blocks (hidden)
MT = N // P       # 64 m-tiles (global node contraction)
HF = 512          # matmul moving free dim (PSUM bank limit)
IH = ROWS // HF   # 2  i-halves of the local row range
OB = OUTD // P    # 2  output-feature blocks
GC = 2            # allgather chunks for both S and Z (one per i-half)
QT = 4            # m-tiles per (chunk, core-block) = IB // GC
TAIL = [(1, 5), (1, 6), (1, 7)]  # (c,k) groups computed on-core
TB = len(TAIL) * QT      # tail i-blocks (12)
NWARM = 24               # PE-warmup matmuls during the DMA preamble

_BF16 = ml_dtypes.bfloat16
_F8 = ml_dtypes.float8_e4m3

_cache = {}


def _build():
    import concourse.mybir as mybir
    import concourse.tile as tile
    from concourse import bacc

    dt = mybir.dt
    f32 = dt.float32
    bf16 = dt.bfloat16
    f8 = dt.float8e4
    AF = mybir.ActivationFunctionType
    DR = mybir.MatmulPerfMode.DoubleRow
    MUL = mybir.AluOpType.mult
    ADD = mybir.AluOpType.add
    MAX = mybir.AluOpType.max

    nc = bacc.Bacc("TRN2", target_bir_lowering=False, debug=False,
                   num_devices=NCORES)

    # packed-pair fp8 inputs (see module docstring for layouts)
    xp_e = nc.dram_tensor("xp", [CP * P, 2 * ROWS], f8, kind="ExternalInput")
    xtp_e = nc.dram_tensor("xtp", [CP * P, 2 * TB * P], f8,
                           kind="ExternalInput")
    ifp_e = nc.dram_tensor("ifp", [IH * (MT // 2) * P, 2 * HF], f8,
                           kind="ExternalInput")
    adjp_e = nc.dram_tensor("adjp", [(MT // 2) * P, 2 * ROWS], f8,
                            kind="ExternalInput")
    w1p_e = nc.dram_tensor("w1p", [CP * P, 2 * NHID], f8,
                           kind="ExternalInput")
    w2_e = nc.dram_tensor("w2", [NHID, OUTD], bf16, kind="ExternalInput")
    b1p_e = nc.dram_tensor("b1p", [P, JB], f32, kind="ExternalInput")
    bnsc_e = nc.dram_tensor("bnsc", [P, OB], f32, kind="ExternalInput")
    bnbi_e = nc.dram_tensor("bnbi", [P, OB], f32, kind="ExternalInput")
    uco_e = nc.dram_tensor("uco", [1, NHID], bf16, kind="ExternalInput")
    wco_e = nc.dram_tensor("wco", [1, ROWS], bf16, kind="ExternalInput")
    # outT: [OUTD, ROWS]; the host transposes each core's block.
    out_e = nc.dram_tensor("out", [OUTD, ROWS], f32, kind="ExternalOutput")

    groups = [list(range(NCORES))]

    def allgather(g_in, g_out):
        nc.gpsimd.collective_compute(
            "AllGather", mybir.AluOpType.bypass, replica_groups=groups,
            ins=[g_in[:]], outs=[g_out[:]])

    with tile.TileContext(nc) as tc:
        with (
            tc.tile_pool(name="const", bufs=1) as const,
            tc.tile_pool(name="sloc", bufs=1) as sloc_p,
            tc.tile_pool(name="h1", bufs=1) as h1_p,
            tc.tile_pool(name="zsb", bufs=1) as z_p,
            tc.tile_pool(name="schunk", bufs=13) as schunk_p,
            tc.tile_pool(name="zchunk", bufs=12) as zchunk_p,
            tc.tile_pool(name="astream", bufs=10) as astream,
            tc.tile_pool(name="afull", bufs=32) as afull_p,
            tc.tile_pool(name="outsb", bufs=1) as outsb_p,
            tc.tile_pool(name="dram", bufs=1, space="DRAM") as dram,
        ):
            # ---- collective warmup: absorb the CC bootstrap/skew barrier
            # concurrently with the DMA preamble (the input only needs a
            # trivial bounce DMA, so this fires almost immediately; the
            # output is never read)
            wu_in = dram.tile([P, JB], f32, name="wui")
            wu_out = dram.tile([NCORES * P, JB], f32, addr_space="Shared",
                               name="wu")
            nc.sync.dma_start(wu_in[:], b1p_e[:])
            allgather(wu_in, wu_out)

            # ---- constants into SBUF (w1/x first: phase 1 needs them)
            w1_sb = const.tile([P, CP, 2, NHID], f8)
            nc.sync.dma_start(
                w1_sb[:],
                w1p_e[:].rearrange("(cp p) (t j) -> p cp t j", p=P, t=2))
            x_sb = const.tile([P, CP, 2, ROWS], f8)
            nc.sync.dma_start(
                x_sb[:],
                xp_e[:].rearrange("(cp p) (t i) -> p cp t i", p=P, t=2))
            b1p_sb = const.tile([P, JB], f32)
            nc.sync.dma_start(b1p_sb[:], b1p_e[:])
            bnsc_sb = const.tile([P, OB], f32)
            nc.sync.dma_start(bnsc_sb[:], bnsc_e[:])
            bnbi_sb = const.tile([P, OB], f32)
            nc.sync.dma_start(bnbi_sb[:], bnbi_e[:])
            uco_sb = const.tile([1, NHID], bf16)
            nc.sync.dma_start(uco_sb[:], uco_e[:])
            wco_sb = const.tile([1, ROWS], bf16)
            nc.sync.dma_start(wco_sb[:], wco_e[:])
            xt_sb = const.tile([P, CP, 2, TB * P], f8)
            nc.sync.dma_start(
                xt_sb[:],
                xtp_e[:].rearrange("(cp p) (t i) -> p cp t i", p=P, t=2))
            w2_sb = const.tile([P, JB, OUTD], bf16)
            nc.sync.dma_start(
                w2_sb[:], w2_e[:].rearrange("(jb p) o -> p jb o", p=P))

            # ---- DRAM bounce buffers for the chunked fp8 collectives
            s_bounce = [dram.tile([2 * P, 2 * NHID], f8, name=f"sb{c}")
                        for c in range(GC)]
            s_all = [dram.tile([2 * P * NCORES, 2 * NHID], f8,
                               addr_space="Shared", name=f"sa{c}")
                     for c in range(GC)]
            z_bounce = [dram.tile([2 * P, 2 * OUTD], f8, name=f"zb{c}")
                        for c in range(GC)]
            z_all = [dram.tile([2 * P * NCORES, 2 * OUTD], f8,
                               addr_space="Shared", name=f"za{c}")
                     for c in range(GC)]

            def s_evict(dst, ps, ib):
                # alternate scalar/vector so back-to-back evictions overlap
                if ib % 2 == 0:
                    nc.scalar.activation(dst, ps[:], AF.Copy, scale=0.125)
                else:
                    nc.vector.tensor_scalar(dst, ps[:], 0.125, 0.0, MUL, ADD)

            # ---- phase 1: S_k = x[R_k] @ W1 in fp8 DR; bounce+gather/chunk
            s_loc = sloc_p.tile([P, IB, NHID], f8)
            with tc.tile_pool(name="ps1", bufs=2, space="PSUM") as ps1:
                # PE warmup: keep the tensor clock ramping while the
                # phase-1 operands stream in (output never read)
                dummy = const.tile([P, 2, HF], f8)
                nc.vector.memset(dummy[:], 0)
                ps_w = ps1.tile([P, HF], f32, tag="warm")
                for _ in range(NWARM):
                    nc.tensor.matmul(ps_w[:], dummy[:, :, 0:P], dummy[:],
                                     start=True, stop=True, perf_mode=DR)
                for c in range(GC):
                    for t in range(QT):
                        ib = c * QT + t
                        ps = ps1.tile([P, NHID], f32, tag="s")
                        for cp in range(CP):
                            nc.tensor.matmul(
                                ps[:],
                                x_sb[:, cp, :, ib * P:(ib + 1) * P],
                                w1_sb[:, cp, :, :],
                                start=(cp == 0), stop=(cp == CP - 1),
                                perf_mode=DR,
                            )
                        s_evict(s_loc[:, ib, :], ps, ib)
                    nc.sync.dma_start(
                        s_bounce[c][:].rearrange(
                            "(qq p) (t j) -> p qq t j", p=P, t=2),
                        s_loc[:, c * QT:(c + 1) * QT, :].rearrange(
                            "p (qq t) j -> p qq t j", qq=2))
                    allgather(s_bounce[c], s_all[c])
                # tail: redundantly compute S rows of the TAIL (c,k) groups
                # locally (identical on every core) so phase 2 has work
                # while the first gather is still in flight
                s_tail = sloc_p.tile([P, TB, NHID], f8)
                for tb in range(TB):
                    ps = ps1.tile([P, NHID], f32, tag="s")
                    for cp in range(CP):
                        nc.tensor.matmul(
                            ps[:],
                            xt_sb[:, cp, :, tb * P:(tb + 1) * P],
                            w1_sb[:, cp, :, :],
                            start=(cp == 0), stop=(cp == CP - 1),
                            perf_mode=DR,
                        )
                    s_evict(s_tail[:, tb, :], ps, tb)

            # gathered-S staging: chunk c, core-block k -> 2 m-tile pairs
            s_sb = [[None] * NCORES for _ in range(GC)]

            def stage_s(c, k):
                tile_ = schunk_p.tile([P, 2, 2, NHID], f8, tag="schunk")
                nc.sync.dma_start(
                    tile_[:],
                    s_all[c][k * 2 * P:(k + 1) * 2 * P, :]
                    .rearrange("(qq p) (t j) -> p qq t j", p=P, t=2))
                s_sb[c][k] = tile_

            h1T = h1_p.tile([P, JB, ROWS], bf16)
            z_sb = z_p.tile([P, IB, OUTD], f8)

            # phase-4 adj stream, prefetched during phase 2 (1 pair-tile
            # per phase-2 group keeps DMA below the HBM ceiling)
            a4_sb = {}
            p4_seq = [(c, k) for c in range(GC) for k in range(NCORES)]
            a4_queue = [(c, k, qq) for c, k in p4_seq for qq in range(2)]

            def load_a4():
                if a4_queue:
                    c, k, qq = a4_queue.pop(0)
                    g = 4 * k + 2 * c + qq
                    t = afull_p.tile([P, 2, ROWS], f8, tag="afull")
                    nc.sync.dma_start(
                        t[:],
                        adjp_e[g * P:(g + 1) * P, :]
                        .rearrange("p (t i) -> p t i", t=2))
                    a4_sb[(c, k, qq)] = t

            # traversal: tail groups first (local S), then chunk-major
            order = TAIL + [(0, k) for k in range(NCORES)] \
                + [(1, k) for k in range(NCORES) if (1, k) not in TAIL]
            staged_seq = [ck for ck in order if ck not in TAIL]

            def a_load(ih, g):
                t = astream.tile([P, 2, HF], f8, tag="ahalf")
                nc.sync.dma_start(
                    t[:],
                    ifp_e[(ih * (MT // 2) + g) * P:
                          (ih * (MT // 2) + g + 1) * P, :]
                    .rearrange("p (t f) -> p t f", t=2))
                return t

            # ---- phase 2+3, i-half pass ih: accumulate h1T half in fp8
            # DR, add the rank-1 correction, emit z half (bf16), and fire
            # the Z allgather chunk for that half mid-kernel.
            def l1_pass(ih, ps2, ps3, pre_a, nxt_a):
                psum_h = [ps2.tile([P, HF], f32, name=f"ph{jb}_{ih}",
                                   tag=f"ph{jb}")
                          for jb in range(JB)]
                n_staged = 0
                n = 0
                for gi, (c, k) in enumerate(order):
                    tail_i = TAIL.index((c, k)) if (c, k) in TAIL else -1
                    for qq in range(2):
                        g = 4 * k + 2 * c + qq
                        key = (ih, g)
                        a_tile = pre_a.pop(key, None)
                        if a_tile is None:
                            a_tile = a_load(ih, g)
                        if tail_i >= 0:
                            s_src = s_tail[:, 4 * tail_i + 2 * qq:
                                           4 * tail_i + 2 * qq + 2, :]
                        else:
                            s_src = s_sb[c][k][:, qq, :, :]
                        for jb in range(JB):
                            nc.tensor.matmul(
                                psum_h[jb][:],
                                s_src[:, :, jb * P:(jb + 1) * P],
                                a_tile[:],
                                start=(n == 0), stop=False,
                                perf_mode=DR,
                            )
                        n += 1
                    # gather-dependent staging strictly AFTER this group's
                    # independent loads (single in-order DMA queue)
                    if ih == 0 and n_staged < len(staged_seq):
                        stage_s(*staged_seq[n_staged])
                        n_staged += 1
                    load_a4()
                # preload the next pass's first a-tiles before the
                # epilogue chain so its start is not DMA-gated
                for c, k in order[:2]:
                    for qq in range(2):
                        g = 4 * k + 2 * c + qq
                        if nxt_a is not None:
                            nxt_a[(ih + 1, g)] = a_load(ih + 1, g)
                # rank-1 correction closes each accumulation group
                for jb in range(JB):
                    nc.tensor.matmul(
                        psum_h[jb][:],
                        uco_sb[:, jb * P:(jb + 1) * P],
                        wco_sb[:, ih * HF:(ih + 1) * HF],
                        start=False, stop=True,
                    )
                # epilogue: relu+bias into h1T half (scalar/vector split)
                for jb in range(JB):
                    dst = h1T[:, jb, ih * HF:(ih + 1) * HF]
                    if jb % 2 == 0:
                        nc.scalar.activation(dst, psum_h[jb][:], AF.Relu,
                                             bias=b1p_sb[:, jb:jb + 1])
                    else:
                        nc.vector.tensor_scalar(dst, psum_h[jb][:],
                                                b1p_sb[:, jb:jb + 1], 0.0,
                                                ADD, MAX)
                # z for this half's i-blocks (bf16), bounce, gather chunk
                for t in range(QT):
                    ib = ih * QT + t
                    ps = ps3.tile([P, OUTD], f32, tag="z")
                    for jb in range(JB):
                        nc.tensor.matmul(
                            ps[:],
                            h1T[:, jb, ib * P:(ib + 1) * P],
                            w2_sb[:, jb, :],
                            start=(jb == 0), stop=(jb == JB - 1),
                        )
                    if t % 2 == 0:
                        nc.scalar.activation(z_sb[:, ib, :], ps[:], AF.Copy)
                    else:
                        nc.vector.tensor_scalar(z_sb[:, ib, :], ps[:],
                                                1.0, 0.0, MUL, ADD)
                nc.sync.dma_start(
                    z_bounce[ih][:].rearrange(
                        "(qq p) (t o) -> p qq t o", p=P, t=2),
                    z_sb[:, ih * QT:(ih + 1) * QT, :].rearrange(
                        "p (qq t) o -> p qq t o", qq=2))
                allgather(z_bounce[ih], z_all[ih])

            with (
                tc.tile_pool(name="ps2", bufs=1, space="PSUM") as ps2,
                tc.tile_pool(name="ps3", bufs=2, space="PSUM") as ps3,
            ):
                handoff = {}
                l1_pass(0, ps2, ps3, {}, handoff)
                l1_pass(1, ps2, ps3, handoff, None)

            # ---- phase 4: outT[o, i] = sum_m Z[m, o] * adjT[m, i] in fp8
            # DR, BN fused on the PSUM evict.  Chunk-major over Z chunks.
            outT_sb = outsb_p.tile([P, OB, ROWS], f32)
            zc_sb = [[None] * NCORES for _ in range(GC)]

            def stage_z(c, k):
                tile_ = zchunk_p.tile([P, 2, 2, OUTD], f8, tag="zchunk")
                nc.sync.dma_start(
                    tile_[:],
                    z_all[c][k * 2 * P:(k + 1) * 2 * P, :]
                    .rearrange("(qq p) (t o) -> p qq t o", p=P, t=2))
                zc_sb[c][k] = tile_

            with tc.tile_pool(name="ps4", bufs=1, space="PSUM") as ps4:
                psum_o = [[ps4.tile([P, HF], f32, name=f"po{ob}_{ih}",
                                    tag=f"po{ob}_{ih}")
                           for ih in range(IH)] for ob in range(OB)]
                # drain any adj tiles not yet prefetched, then stage all
                # z chunks (c=0 is already gathered; c=1 staging may wait
                # on the second Z gather and blocks only the output DMA)
                while a4_queue:
                    load_a4()
                for k in range(NCORES):
                    stage_z(0, k)
                for k in range(NCORES):
                    stage_z(1, k)
                first = True
                for c, k in p4_seq:
                    zc = zc_sb[c][k]
                    final_grp = (c == GC - 1 and k == NCORES - 1)
                    if not final_grp:
                        for qq in range(2):
                            a_tile = a4_sb[(c, k, qq)]
                            for ob in range(OB):
                                for ihh in range(IH):
                                    nc.tensor.matmul(
                                        psum_o[ob][ihh][:],
                                        zc[:, qq, :, ob * P:(ob + 1) * P],
                                        a_tile[:, :,
                                               ihh * HF:(ihh + 1) * HF],
                                        start=first, stop=False,
                                        perf_mode=DR,
                                    )
                            first = False
                    else:
                        # last group: finish ob=0's accumulators first so
                        # their eviction overlaps ob=1's final matmuls
                        for ob in range(OB):
                            for qq in range(2):
                                a_tile = a4_sb[(c, k, qq)]
                                for ihh in range(IH):
                                    nc.tensor.matmul(
                                        psum_o[ob][ihh][:],
                                        zc[:, qq, :, ob * P:(ob + 1) * P],
                                        a_tile[:, :,
                                               ihh * HF:(ihh + 1) * HF],
                                        start=False, stop=(qq == 1),
                                        perf_mode=DR,
                                    )
                # fused BN affine on PSUM evict: out = psum*scale + bias
                for ob in range(OB):
                    for ihh in range(IH):
                        nc.vector.tensor_scalar(
                            outT_sb[:, ob, ihh * HF:(ihh + 1) * HF],
                            psum_o[ob][ihh][:],
                            bnsc_sb[:, ob:ob + 1],
                            bnbi_sb[:, ob:ob + 1],
                            MUL, ADD)
                    nc.sync.dma_start(
                        out_e[ob * P:(ob + 1) * P, :], outT_sb[:, ob, :])

    nc.compile()
    return nc


def _get_nc():
    if "nc" not in _cache:
        _cache["nc"] = _build()
    return _cache["nc"]


def _pack_pairs(mat_kx, width):
    """[K, width] -> packed [K/256*128, 2*width]: row g*128+p holds the
    DoubleRow pair's two k-tiles (rows 2g*128+p and (2g+1)*128+p)."""
    k = mat_kx.shape[0]
    return np.ascontiguousarray(
        mat_kx.reshape(k // 256, 2, P, width)
        .transpose(0, 2, 1, 3).reshape(k // 2, 2 * width))


def kernel(x, IFadj, adj, W1, b1, W2, b2, bn_gamma, bn_beta, bn_mean, bn_var):
    from concourse.bass_utils import run_bass_kernel_spmd

    x = np.asarray(x, dtype=np.float32)
    IFadj = np.asarray(IFadj, dtype=np.float32)
    adj = np.asarray(adj, dtype=np.float32)
    W1 = np.asarray(W1, dtype=np.float32)
    b1 = np.asarray(b1, dtype=np.float32)
    W2 = np.asarray(W2, dtype=np.float32)
    b2 = np.asarray(b2, dtype=np.float32)
    bn_gamma = np.asarray(bn_gamma, dtype=np.float32)
    bn_beta = np.asarray(bn_beta, dtype=np.float32)
    bn_mean = np.asarray(bn_mean, dtype=np.float32)
    bn_var = np.asarray(bn_var, dtype=np.float32)

    # host-side prep: fp8 casts, DoubleRow pair packing, correction vecs
    x8 = x.astype(_F8)
    W18 = (8.0 * W1).astype(_F8)
    w1p = _pack_pairs(W18, NHID)
    w2b = W2.astype(_BF16)
    b1p = np.ascontiguousarray(b1.reshape(JB, P).T)  # [P, JB]
    inv = bn_gamma / np.sqrt(bn_var + BN_EPS)
    bias_tot = b2 * inv + bn_beta - bn_mean * inv
    bnsc = np.ascontiguousarray(inv.reshape(OB, P).T)       # [P, OB]
    bnbi = np.ascontiguousarray(bias_tot.reshape(OB, P).T)  # [P, OB]

    # rank-1 correction: u = colsum(S_true) - colsum(S8_device-replica)
    S_host = (x8.astype(np.float32) @ W18.astype(np.float32)) * 0.125
    S8 = S_host.astype(_F8)
    u = (x @ W1).sum(0) - S8.astype(np.float32).sum(0)
    uco = np.ascontiguousarray(u.astype(_BF16).reshape(1, NHID))

    IFadj8 = IFadj.astype(_F8)
    adj8 = adj.astype(_F8)
    abar = IFadj8.astype(np.float32).sum(1) / float(N)  # [N]

    # x rows for the TAIL m-tiles (c,k) in TAIL -> mt = 8k+4c+q
    tail_rows = np.concatenate(
        [x8[(8 * k + 4 * c) * P:(8 * k + 4 * c + QT) * P]
         for c, k in TAIL])
    xtp = _pack_pairs(np.ascontiguousarray(tail_rows.T), TB * P)

    in_maps = []
    for k in range(NCORES):
        r0, r1 = k * ROWS, (k + 1) * ROWS
        ifT = np.ascontiguousarray(IFadj8[r0:r1].T)  # [N, ROWS]
        # per-ih-half pair packing: row (ih*4096 + g*128 + p)
        ifp = np.ascontiguousarray(
            ifT.reshape(MT // 2, 2, P, IH, HF)
            .transpose(3, 0, 2, 1, 4).reshape(IH * (MT // 2) * P, 2 * HF))
        adjp = _pack_pairs(np.ascontiguousarray(adj8[r0:r1].T), ROWS)
        xp = _pack_pairs(np.ascontiguousarray(x8[r0:r1].T), ROWS)
        wco = np.ascontiguousarray(abar[r0:r1].astype(_BF16).reshape(1, ROWS))
        in_maps.append({
            "xp": xp,
            "xtp": xtp,
            "ifp": ifp,
            "adjp": adjp,
            "w1p": w1p,
            "w2": w2b,
            "b1p": b1p,
            "bnsc": bnsc,
            "bnbi": bnbi,
            "uco": uco,
            "wco": wco,
        })

    global _last_in_maps
    _last_in_maps = in_maps

    nc = _get_nc()
    try:
        res = run_bass_kernel_spmd(nc, in_maps, list(range(NCORES)))
    except Exception:
        # transient device wedge (NRT_EXEC_UNIT_UNRECOVERABLE etc.) --
        # a straight retry has been observed to recover
        import time
        time.sleep(2.0)
        res = run_bass_kernel_spmd(nc, in_maps, list(range(NCORES)))
    # per-core output is outT [OUTD, ROWS]; transpose back and stack rows
    return np.concatenate(
        [np.ascontiguousarray(res.results[k]["out"].T)
         for k in range(NCORES)], axis=0)


# revision 12
# speedup vs baseline: 1.0608x; 1.0608x over previous
"""Trainium2 Bass kernel for a 2-layer DGCN (graph conv) on 8 NeuronCores.

Reference computation (fp32):
    h1  = relu(IFadj @ (x @ W1) + b1)         # [N, NHID]
    out = BN(adj @ (h1 @ W2) + b2)            # [N, OUTD], BN in eval mode

Distribution: rows of x / IFadj / adj are sharded across 8 cores
(row-parallel graph partitioning).  Matmul phases 1, 2, 4 run in fp8
e4m3 DoubleRow mode (two 128-deep k-tiles per instruction, 2x the bf16
rate); phase 3 (h1 @ W2) stays bf16 because W2-quantization error is
coherent across nodes (h1 has a positive mean) and would be amplified
by adj's mean in phase 4.

fp8 accuracy: the dominant quantization error is the column-sum of the
S = x@W1 residual, amplified by the means of BOTH IFadj and adj.  That
error is rank-1 -- rowsum(IFadj8)/N x colsum(S_true - S8) -- and is
cancelled by ONE extra K=1 matmul accumulated into each phase-2 PSUM
group; the u vector is computed on the host (it also absorbs phase-1's
own fp8 error).  W1 is pre-scaled by 8 (and S by 1/8 on the PSUM
evict) to keep everything in e4m3's normal range.

Latency structure (all DMAs share one in-order DGE queue, collectives
have ~16 us fixed latency, and the PE clock drops to 1.2 GHz after any
idle gap):
  - a dummy AllGather on b1p is issued first so the collective
    bootstrap/skew barrier runs concurrently with the DMA preamble;
  - dummy matmuls on a zeroed tile keep the PE busy (and its clock
    ramping) while the phase-1 operands stream in;
  - gather-dependent staging DMAs are issued strictly after the
    independent loads they would otherwise block in the queue;
  - phase-4's adj stream is prefetched during phase 2 so phase 4 is
    not DMA-bound;
  - each core redundantly computes the S rows of 4 (chunk,core) groups
    (the "tail") and phase 2 consumes those first, covering the S
    allgather latency.
"""

import numpy as np
import ml_dtypes

NCORES = 8
N = 8192
NFEAT = 1024
NHID = 512
OUTD = 256
ROWS = N // NCORES  # 1024
P = 128
BN_EPS = 1e-5

CB = NFEAT // P   # 8  c-blocks (x feature contraction)
CP = CB // 2      # 4  c-pairs (DoubleRow)
IB = ROWS // P    # 8  i-blocks (local rows)
JB = NHID // P    # 4  j-blocks (hidden)
MT = N // P       # 64 m-tiles (global node contraction)
HF = 512          # matmul moving free dim (PSUM bank limit)
IH = ROWS // HF   # 2  i-halves of the local row range
OB = OUTD // P    # 2  output-feature blocks
GC = 2            # allgather chunks for both S and Z (one per i-half)
QT = 4            # m-tiles per (chunk, core-block) = IB // GC
TAIL = [(1, 5), (1, 6), (1, 7)]  # (c,k) groups computed on-core
TB = len(TAIL) * QT      # tail i-blocks (12)
NWARM = 24               # PE-warmup matmuls during the DMA preamble

_BF16 = ml_dtypes.bfloat16
_F8 = ml_dtypes.float8_e4m3

_cache = {}


def _build():
    import concourse.mybir as mybir
    import concourse.tile as tile
    from concourse import bacc

    dt = mybir.dt
    f32 = dt.float32
    bf16 = dt.bfloat16
    f8 = dt.float8e4
    AF = mybir.ActivationFunctionType
    DR = mybir.MatmulPerfMode.DoubleRow
    MUL = mybir.AluOpType.mult
    ADD = mybir.AluOpType.add
    MAX = mybir.AluOpType.max

    nc = bacc.Bacc("TRN2", target_bir_lowering=False, debug=False,
                   num_devices=NCORES)

    # packed-pair fp8 inputs (see module docstring for layouts)
    xp_e = nc.dram_tensor("xp", [2 * CP * P, ROWS], f8, kind="ExternalInput")
    xtp_e = nc.dram_tensor("xtp", [CP * P, 2 * TB * P], f8,
                           kind="ExternalInput")
    ifp_e = nc.dram_tensor("ifp", [IH * (MT // 2) * P, 2 * HF], f8,
                           kind="ExternalInput")
    adjp_e = nc.dram_tensor("adjp", [(MT // 2) * P, 2 * ROWS], f8,
                            kind="ExternalInput")
    w1p_e = nc.dram_tensor("w1p", [CP * P, 2 * NHID], f8,
                           kind="ExternalInput")
    w2_e = nc.dram_tensor("w2", [NHID, OUTD], bf16, kind="ExternalInput")
    b1p_e = nc.dram_tensor("b1p", [P, JB], f32, kind="ExternalInput")
    bnsc_e = nc.dram_tensor("bnsc", [P, OB], f32, kind="ExternalInput")
    bnbi_e = nc.dram_tensor("bnbi", [P, OB], f32, kind="ExternalInput")
    uco_e = nc.dram_tensor("uco", [1, NHID], bf16, kind="ExternalInput")
    wco_e = nc.dram_tensor("wco", [1, ROWS], bf16, kind="ExternalInput")
    # outT: [OUTD, ROWS]; the host transposes each core's block.
    out_e = nc.dram_tensor("out", [OUTD, ROWS], f32, kind="ExternalOutput")

    groups = [list(range(NCORES))]

    def allgather(g_in, g_out):
        nc.gpsimd.collective_compute(
            "AllGather", mybir.AluOpType.bypass, replica_groups=groups,
            ins=[g_in[:]], outs=[g_out[:]])

    with tile.TileContext(nc) as tc:
        with (
            tc.tile_pool(name="const", bufs=1) as const,
            tc.tile_pool(name="sloc", bufs=1) as sloc_p,
            tc.tile_pool(name="h1", bufs=1) as h1_p,
            tc.tile_pool(name="zsb", bufs=1) as z_p,
            tc.tile_pool(name="schunk", bufs=13) as schunk_p,
            tc.tile_pool(name="zchunk", bufs=12) as zchunk_p,
            tc.tile_pool(name="astream", bufs=10) as astream,
            tc.tile_pool(name="afull", bufs=32) as afull_p,
            tc.tile_pool(name="outsb", bufs=1) as outsb_p,
            tc.tile_pool(name="dram", bufs=1, space="DRAM") as dram,
        ):
            # ---- constants into SBUF (w1/x first: phase 1 needs them;
            # x in halves so S-chunk-0 production waits on only 1 MB)
            w1_sb = const.tile([P, CP, 2, NHID], f8)
            nc.sync.dma_start(
                w1_sb[:],
                w1p_e[:].rearrange("(cp p) (t j) -> p cp t j", p=P, t=2))
            x_sb = []
            for h in range(2):
                xh = const.tile([P, CP, 2, ROWS // 2], f8)
                nc.sync.dma_start(
                    xh[:],
                    xp_e[h * CP * P:(h + 1) * CP * P, :].rearrange(
                        "(cp p) (t i) -> p cp t i", p=P, t=2))
                x_sb.append(xh)
            b1p_sb = const.tile([P, JB], f32)
            nc.sync.dma_start(b1p_sb[:], b1p_e[:])
            bnsc_sb = const.tile([P, OB], f32)
            nc.sync.dma_start(bnsc_sb[:], bnsc_e[:])
            bnbi_sb = const.tile([P, OB], f32)
            nc.sync.dma_start(bnbi_sb[:], bnbi_e[:])
            uco_sb = const.tile([1, NHID], bf16)
            nc.sync.dma_start(uco_sb[:], uco_e[:])
            wco_sb = const.tile([1, ROWS], bf16)
            nc.sync.dma_start(wco_sb[:], wco_e[:])
            xt_sb = const.tile([P, CP, 2, TB * P], f8)
            nc.sync.dma_start(
                xt_sb[:],
                xtp_e[:].rearrange("(cp p) (t i) -> p cp t i", p=P, t=2))
            w2_sb = const.tile([P, JB, OUTD], bf16)
            nc.sync.dma_start(
                w2_sb[:], w2_e[:].rearrange("(jb p) o -> p jb o", p=P))

            # ---- DRAM bounce buffers for the chunked fp8 collectives
            s_bounce = [dram.tile([2 * P, 2 * NHID], f8, name=f"sb{c}")
                        for c in range(GC)]
            s_all = [dram.tile([2 * P * NCORES, 2 * NHID], f8,
                               addr_space="Shared", name=f"sa{c}")
                     for c in range(GC)]
            z_bounce = [dram.tile([2 * P, 2 * OUTD], f8, name=f"zb{c}")
                        for c in range(GC)]
            z_all = [dram.tile([2 * P * NCORES, 2 * OUTD], f8,
                               addr_space="Shared", name=f"za{c}")
                     for c in range(GC)]

            def s_evict(dst, ps, ib):
                # alternate scalar/vector so back-to-back evictions overlap
                if ib % 2 == 0:
                    nc.scalar.activation(dst, ps[:], AF.Copy, scale=0.125)
                else:
                    nc.vector.tensor_scalar(dst, ps[:], 0.125, 0.0, MUL, ADD)

            # ---- phase 1: S_k = x[R_k] @ W1 in fp8 DR; bounce+gather/chunk
            s_loc = sloc_p.tile([P, IB, NHID], f8)
            with tc.tile_pool(name="ps1", bufs=2, space="PSUM") as ps1:
                # PE warmup: keep the tensor clock ramping while the
                # phase-1 operands stream in (output never read)
                dummy = const.tile([P, 2, HF], f8)
                nc.vector.memset(dummy[:], 0)
                ps_w = ps1.tile([P, HF], f32, tag="warm")
                for _ in range(NWARM):
                    nc.tensor.matmul(ps_w[:], dummy[:, :, 0:P], dummy[:],
                                     start=True, stop=True, perf_mode=DR)
                for c in range(GC):
                    for t in range(QT):
                        ib = c * QT + t
                        ps = ps1.tile([P, NHID], f32, tag="s")
                        for cp in range(CP):
                            nc.tensor.matmul(
                                ps[:],
                                x_sb[ib // QT][:, cp, :,
                                               (ib % QT) * P:
                                               (ib % QT + 1) * P],
                                w1_sb[:, cp, :, :],
                                start=(cp == 0), stop=(cp == CP - 1),
                                perf_mode=DR,
                            )
                        s_evict(s_loc[:, ib, :], ps, ib)
                    nc.sync.dma_start(
                        s_bounce[c][:].rearrange(
                            "(qq p) (t j) -> p qq t j", p=P, t=2),
                        s_loc[:, c * QT:(c + 1) * QT, :].rearrange(
                            "p (qq t) j -> p qq t j", qq=2))
                    allgather(s_bounce[c], s_all[c])
                # tail: redundantly compute S rows of the TAIL (c,k) groups
                # locally (identical on every core) so phase 2 has work
                # while the first gather is still in flight
                s_tail = sloc_p.tile([P, TB, NHID], f8)
                for tb in range(TB):
                    ps = ps1.tile([P, NHID], f32, tag="s")
                    for cp in range(CP):
                        nc.tensor.matmul(
                            ps[:],
                            xt_sb[:, cp, :, tb * P:(tb + 1) * P],
                            w1_sb[:, cp, :, :],
                            start=(cp == 0), stop=(cp == CP - 1),
                            perf_mode=DR,
                        )
                    s_evict(s_tail[:, tb, :], ps, tb)

            # gathered-S staging: chunk c, core-block k -> 2 m-tile pairs
            s_sb = [[None] * NCORES for _ in range(GC)]

            def stage_s(c, k):
                tile_ = schunk_p.tile([P, 2, 2, NHID], f8, tag="schunk")
                nc.sync.dma_start(
                    tile_[:],
                    s_all[c][k * 2 * P:(k + 1) * 2 * P, :]
                    .rearrange("(qq p) (t j) -> p qq t j", p=P, t=2))
                s_sb[c][k] = tile_

            h1T = h1_p.tile([P, JB, ROWS], bf16)
            z_sb = z_p.tile([P, IB, OUTD], f8)

            # phase-4 adj stream, prefetched during phase 2 (1 pair-tile
            # per phase-2 group keeps DMA below the HBM ceiling)
            a4_sb = {}
            p4_seq = [(c, k) for c in range(GC) for k in range(NCORES)]
            a4_queue = [(c, k, qq) for c, k in p4_seq for qq in range(2)]

            def load_a4():
                if a4_queue:
                    c, k, qq = a4_queue.pop(0)
                    g = 4 * k + 2 * c + qq
                    t = afull_p.tile([P, 2, ROWS], f8, tag="afull")
                    nc.sync.dma_start(
                        t[:],
                        adjp_e[g * P:(g + 1) * P, :]
                        .rearrange("p (t i) -> p t i", t=2))
                    a4_sb[(c, k, qq)] = t

            # traversal: tail groups first (local S), then chunk-major
            order = TAIL + [(0, k) for k in range(NCORES)] \
                + [(1, k) for k in range(NCORES) if (1, k) not in TAIL]
            staged_seq = [ck for ck in order if ck not in TAIL]

            def a_load(ih, g):
                t = astream.tile([P, 2, HF], f8, tag="ahalf")
                nc.sync.dma_start(
                    t[:],
                    ifp_e[(ih * (MT // 2) + g) * P:
                          (ih * (MT // 2) + g + 1) * P, :]
                    .rearrange("p (t f) -> p t f", t=2))
                return t

            # ---- phase 2+3, i-half pass ih: accumulate h1T half in fp8
            # DR, add the rank-1 correction, emit z half (bf16), and fire
            # the Z allgather chunk for that half mid-kernel.
            def l1_pass(ih, ps2, ps3, pre_a, nxt_a):
                psum_h = [ps2.tile([P, HF], f32, name=f"ph{jb}_{ih}",
                                   tag=f"ph{jb}")
                          for jb in range(JB)]
                n_staged = 0
                n = 0
                for gi, (c, k) in enumerate(order):
                    tail_i = TAIL.index((c, k)) if (c, k) in TAIL else -1
                    for qq in range(2):
                        g = 4 * k + 2 * c + qq
                        key = (ih, g)
                        a_tile = pre_a.pop(key, None)
                        if a_tile is None:
                            a_tile = a_load(ih, g)
                        if tail_i >= 0:
                            s_src = s_tail[:, 4 * tail_i + 2 * qq:
                                           4 * tail_i + 2 * qq + 2, :]
                        else:
                            s_src = s_sb[c][k][:, qq, :, :]
                        for jb in range(JB):
                            nc.tensor.matmul(
                                psum_h[jb][:],
                                s_src[:, :, jb * P:(jb + 1) * P],
                                a_tile[:],
                                start=(n == 0), stop=False,
                                perf_mode=DR,
                            )
                        n += 1
                    # gather-dependent staging strictly AFTER this group's
                    # independent loads (single in-order DMA queue)
                    if ih == 0 and n_staged < len(staged_seq):
                        stage_s(*staged_seq[n_staged])
                        n_staged += 1
                    load_a4()
                # preload the next pass's first a-tiles before the
                # epilogue chain so its start is not DMA-gated
                for c, k in order[:2]:
                    for qq in range(2):
                        g = 4 * k + 2 * c + qq
                        if nxt_a is not None:
                            nxt_a[(ih + 1, g)] = a_load(ih + 1, g)
                # rank-1 correction closes each accumulation group
                for jb in range(JB):
                    nc.tensor.matmul(
                        psum_h[jb][:],
                        uco_sb[:, jb * P:(jb + 1) * P],
                        wco_sb[:, ih * HF:(ih + 1) * HF],
                        start=False, stop=True,
                    )
                # epilogue: relu+bias into h1T half (scalar/vector split)
                for jb in range(JB):
                    dst = h1T[:, jb, ih * HF:(ih + 1) * HF]
                    if jb % 2 == 0:
                        nc.scalar.activation(dst, psum_h[jb][:], AF.Relu,
                                             bias=b1p_sb[:, jb:jb + 1])
                    else:
                        nc.vector.tensor_scalar(dst, psum_h[jb][:],
                                                b1p_sb[:, jb:jb + 1], 0.0,
                                                ADD, MAX)
                # z for this half's i-blocks (bf16), bounce, gather chunk
                for t in range(QT):
                    ib = ih * QT + t
                    ps = ps3.tile([P, OUTD], f32, tag="z")
                    for jb in range(JB):
                        nc.tensor.matmul(
                            ps[:],
                            h1T[:, jb, ib * P:(ib + 1) * P],
                            w2_sb[:, jb, :],
                            start=(jb == 0), stop=(jb == JB - 1),
                        )
                    if t % 2 == 0:
                        nc.scalar.activation(z_sb[:, ib, :], ps[:], AF.Copy)
                    else:
                        nc.vector.tensor_scalar(z_sb[:, ib, :], ps[:],
                                                1.0, 0.0, MUL, ADD)
                nc.sync.dma_start(
                    z_bounce[ih][:].rearrange(
                        "(qq p) (t o) -> p qq t o", p=P, t=2),
                    z_sb[:, ih * QT:(ih + 1) * QT, :].rearrange(
                        "p (qq t) o -> p qq t o", qq=2))
                allgather(z_bounce[ih], z_all[ih])

            with (
                tc.tile_pool(name="ps2", bufs=1, space="PSUM") as ps2,
                tc.tile_pool(name="ps3", bufs=2, space="PSUM") as ps3,
            ):
                handoff = {}
                l1_pass(0, ps2, ps3, {}, handoff)
                l1_pass(1, ps2, ps3, handoff, None)

            # ---- phase 4: outT[o, i] = sum_m Z[m, o] * adjT[m, i] in fp8
            # DR, BN fused on the PSUM evict.  Chunk-major over Z chunks.
            outT_sb = outsb_p.tile([P, OB, ROWS], f32)
            zc_sb = [[None] * NCORES for _ in range(GC)]

            def stage_z(c, k):
                tile_ = zchunk_p.tile([P, 2, 2, OUTD], f8, tag="zchunk")
                nc.sync.dma_start(
                    tile_[:],
                    z_all[c][k * 2 * P:(k + 1) * 2 * P, :]
                    .rearrange("(qq p) (t o) -> p qq t o", p=P, t=2))
                zc_sb[c][k] = tile_

            with tc.tile_pool(name="ps4", bufs=1, space="PSUM") as ps4:
                psum_o = [[ps4.tile([P, HF], f32, name=f"po{ob}_{ih}",
                                    tag=f"po{ob}_{ih}")
                           for ih in range(IH)] for ob in range(OB)]
                # drain any adj tiles not yet prefetched, then stage all
                # z chunks (c=0 is already gathered; c=1 staging may wait
                # on the second Z gather and blocks only the output DMA)
                while a4_queue:
                    load_a4()
                for k in range(NCORES):
                    stage_z(0, k)
                for k in range(NCORES):
                    stage_z(1, k)
                first = True
                for c, k in p4_seq:
                    zc = zc_sb[c][k]
                    final_grp = (c == GC - 1 and k == NCORES - 1)
                    if not final_grp:
                        for qq in range(2):
                            a_tile = a4_sb[(c, k, qq)]
                            for ob in range(OB):
                                for ihh in range(IH):
                                    nc.tensor.matmul(
                                        psum_o[ob][ihh][:],
                                        zc[:, qq, :, ob * P:(ob + 1) * P],
                                        a_tile[:, :,
                                               ihh * HF:(ihh + 1) * HF],
                                        start=first, stop=False,
                                        perf_mode=DR,
                                    )
                            first = False
                    else:
                        # last group: finish ob=0's accumulators first so
                        # their eviction overlaps ob=1's final matmuls
                        for ob in range(OB):
                            for qq in range(2):
                                a_tile = a4_sb[(c, k, qq)]
                                for ihh in range(IH):
                                    nc.tensor.matmul(
                                        psum_o[ob][ihh][:],
                                        zc[:, qq, :, ob * P:(ob + 1) * P],
                                        a_tile[:, :,
                                               ihh * HF:(ihh + 1) * HF],
                                        start=False, stop=(qq == 1),
                                        perf_mode=DR,
                                    )
                # fused BN affine on PSUM evict: out = psum*scale + bias
                for ob in range(OB):
                    for ihh in range(IH):
                        nc.vector.tensor_scalar(
                            outT_sb[:, ob, ihh * HF:(ihh + 1) * HF],
                            psum_o[ob][ihh][:],
                            bnsc_sb[:, ob:ob + 1],
                            bnbi_sb[:, ob:ob + 1],
                            MUL, ADD)
                    nc.sync.dma_start(
                        out_e[ob * P:(ob + 1) * P, :], outT_sb[:, ob, :])

    nc.compile()
    return nc


def _get_nc():
    if "nc" not in _cache:
        _cache["nc"] = _build()
    return _cache["nc"]


def _pack_pairs(mat_kx, width):
    """[K, width] -> packed [K/256*128, 2*width]: row g*128+p holds the
    DoubleRow pair's two k-tiles (rows 2g*128+p and (2g+1)*128+p)."""
    k = mat_kx.shape[0]
    return np.ascontiguousarray(
        mat_kx.reshape(k // 256, 2, P, width)
        .transpose(0, 2, 1, 3).reshape(k // 2, 2 * width))


def kernel(x, IFadj, adj, W1, b1, W2, b2, bn_gamma, bn_beta, bn_mean, bn_var):
    from concourse.bass_utils import run_bass_kernel_spmd

    x = np.asarray(x, dtype=np.float32)
    IFadj = np.asarray(IFadj, dtype=np.float32)
    adj = np.asarray(adj, dtype=np.float32)
    W1 = np.asarray(W1, dtype=np.float32)
    b1 = np.asarray(b1, dtype=np.float32)
    W2 = np.asarray(W2, dtype=np.float32)
    b2 = np.asarray(b2, dtype=np.float32)
    bn_gamma = np.asarray(bn_gamma, dtype=np.float32)
    bn_beta = np.asarray(bn_beta, dtype=np.float32)
    bn_mean = np.asarray(bn_mean, dtype=np.float32)
    bn_var = np.asarray(bn_var, dtype=np.float32)

    # host-side prep: fp8 casts, DoubleRow pair packing, correction vecs
    x8 = x.astype(_F8)
    W18 = (8.0 * W1).astype(_F8)
    w1p = _pack_pairs(W18, NHID)
    w2b = W2.astype(_BF16)
    b1p = np.ascontiguousarray(b1.reshape(JB, P).T)  # [P, JB]
    inv = bn_gamma / np.sqrt(bn_var + BN_EPS)
    bias_tot = b2 * inv + bn_beta - bn_mean * inv
    bnsc = np.ascontiguousarray(inv.reshape(OB, P).T)       # [P, OB]
    bnbi = np.ascontiguousarray(bias_tot.reshape(OB, P).T)  # [P, OB]

    # rank-1 correction: u = colsum(S_true) - colsum(S8_device-replica)
    S_host = (x8.astype(np.float32) @ W18.astype(np.float32)) * 0.125
    S8 = S_host.astype(_F8)
    u = (x @ W1).sum(0) - S8.astype(np.float32).sum(0)
    uco = np.ascontiguousarray(u.astype(_BF16).reshape(1, NHID))

    IFadj8 = IFadj.astype(_F8)
    adj8 = adj.astype(_F8)
    abar = IFadj8.astype(np.float32).sum(1) / float(N)  # [N]

    # x rows for the TAIL m-tiles (c,k) in TAIL -> mt = 8k+4c+q
    tail_rows = np.concatenate(
        [x8[(8 * k + 4 * c) * P:(8 * k + 4 * c + QT) * P]
         for c, k in TAIL])
    xtp = _pack_pairs(np.ascontiguousarray(tail_rows.T), TB * P)

    in_maps = []
    for k in range(NCORES):
        r0, r1 = k * ROWS, (k + 1) * ROWS
        ifT = np.ascontiguousarray(IFadj8[r0:r1].T)  # [N, ROWS]
        # per-ih-half pair packing: row (ih*4096 + g*128 + p)
        ifp = np.ascontiguousarray(
            ifT.reshape(MT // 2, 2, P, IH, HF)
            .transpose(3, 0, 2, 1, 4).reshape(IH * (MT // 2) * P, 2 * HF))
        adjp = _pack_pairs(np.ascontiguousarray(adj8[r0:r1].T), ROWS)
        # x packed per i-half: rows [h*512 + cp*128 + p], cols (t, i')
        xp = np.concatenate(
            [_pack_pairs(np.ascontiguousarray(
                x8[r0 + h * (ROWS // 2):r0 + (h + 1) * (ROWS // 2)].T),
                ROWS // 2)
             for h in range(2)], axis=0)
        wco = np.ascontiguousarray(abar[r0:r1].astype(_BF16).reshape(1, ROWS))
        in_maps.append({
            "xp": xp,
            "xtp": xtp,
            "ifp": ifp,
            "adjp": adjp,
            "w1p": w1p,
            "w2": w2b,
            "b1p": b1p,
            "bnsc": bnsc,
            "bnbi": bnbi,
            "uco": uco,
            "wco": wco,
        })

    global _last_in_maps
    _last_in_maps = in_maps

    nc = _get_nc()
    try:
        res = run_bass_kernel_spmd(nc, in_maps, list(range(NCORES)))
    except Exception:
        # transient device wedge (NRT_EXEC_UNIT_UNRECOVERABLE etc.) --
        # a straight retry has been observed to recover
        import time
        time.sleep(2.0)
        res = run_bass_kernel_spmd(nc, in_maps, list(range(NCORES)))
    # per-core output is outT [OUTD, ROWS]; transpose back and stack rows
    return np.concatenate(
        [np.ascontiguousarray(res.results[k]["out"].T)
         for k in range(NCORES)], axis=0)


# revision 16
# speedup vs baseline: 1.1152x; 1.0513x over previous
"""Trainium2 Bass kernel for a 2-layer DGCN (graph conv) on 8 NeuronCores.

Reference computation (fp32):
    h1  = relu(IFadj @ (x @ W1) + b1)         # [N, NHID]
    out = BN(adj @ (h1 @ W2) + b2)            # [N, OUTD], BN in eval mode

Distribution: rows of x / IFadj / adj are sharded across 8 cores
(row-parallel graph partitioning).  Matmul phases 1, 2, 4 run in fp8
e4m3 DoubleRow mode (two 128-deep k-tiles per instruction, 2x the bf16
rate); phase 3 (h1 @ W2) stays bf16 because W2-quantization error is
coherent across nodes (h1 has a positive mean) and would be amplified
by adj's mean in phase 4.

fp8 accuracy: the dominant quantization error is the column-sum of the
S = x@W1 residual, amplified by the means of BOTH IFadj and adj.  That
error is rank-1 -- rowsum(IFadj8)/N x colsum(S_true - S8) -- and is
cancelled by ONE extra K=1 matmul accumulated into each phase-2 PSUM
group; the u vector is computed on the host (it also absorbs phase-1's
own fp8 error).  W1 is pre-scaled by 8 (and S by 1/8 on the PSUM
evict) to keep everything in e4m3's normal range.

Latency structure (all DMAs share one in-order DGE queue, collectives
have ~16 us fixed latency, and the PE clock drops to 1.2 GHz after any
idle gap):
  - a dummy AllGather on b1p is issued first so the collective
    bootstrap/skew barrier runs concurrently with the DMA preamble;
  - dummy matmuls on a zeroed tile keep the PE busy (and its clock
    ramping) while the phase-1 operands stream in;
  - gather-dependent staging DMAs are issued strictly after the
    independent loads they would otherwise block in the queue;
  - phase-4's adj stream is prefetched during phase 2 so phase 4 is
    not DMA-bound;
  - each core redundantly computes the S rows of 4 (chunk,core) groups
    (the "tail") and phase 2 consumes those first, covering the S
    allgather latency.
"""

import numpy as np
import ml_dtypes

NCORES = 8
N = 8192
NFEAT = 1024
NHID = 512
OUTD = 256
ROWS = N // NCORES  # 1024
P = 128
BN_EPS = 1e-5

CB = NFEAT // P   # 8  c-blocks (x feature contraction)
CP = CB // 2      # 4  c-pairs (DoubleRow)
IB = ROWS // P    # 8  i-blocks (local rows)
JB = NHID // P    # 4  j-blocks (hidden)
MT = N // P       # 64 m-tiles (global node contraction)
HF = 512          # matmul moving free dim (PSUM bank limit)
IH = ROWS // HF   # 2  i-halves of the local row range
OB = OUTD // P    # 2  output-feature blocks
GC = 2            # allgather chunks for both S and Z (one per i-half)
QT = 4            # m-tiles per (chunk, core-block) = IB // GC
TAIL = [(1, 5), (1, 6), (1, 7)]  # (c,k) groups computed on-core
TB = len(TAIL) * QT      # tail i-blocks (12)
NWARM = 16               # PE-warmup matmuls during the DMA preamble

_BF16 = ml_dtypes.bfloat16
_F8 = ml_dtypes.float8_e4m3

_cache = {}


def _build():
    import concourse.mybir as mybir
    import concourse.tile as tile
    from concourse import bacc

    dt = mybir.dt
    f32 = dt.float32
    bf16 = dt.bfloat16
    f8 = dt.float8e4
    AF = mybir.ActivationFunctionType
    DR = mybir.MatmulPerfMode.DoubleRow
    MUL = mybir.AluOpType.mult
    ADD = mybir.AluOpType.add
    MAX = mybir.AluOpType.max

    nc = bacc.Bacc("TRN2", target_bir_lowering=False, debug=False,
                   num_devices=NCORES)

    # packed-pair fp8 inputs (see module docstring for layouts)
    xp_e = nc.dram_tensor("xp", [2 * CP * P, ROWS], f8, kind="ExternalInput")
    xtp_e = nc.dram_tensor("xtp", [CP * P, 2 * TB * P], f8,
                           kind="ExternalInput")
    ifp_e = nc.dram_tensor("ifp", [IH * (MT // 2) * P, 2 * HF], f8,
                           kind="ExternalInput")
    adjp_e = nc.dram_tensor("adjp", [(MT // 2) * P, 2 * ROWS], f8,
                            kind="ExternalInput")
    w1p_e = nc.dram_tensor("w1p", [CP * P, 2 * NHID], f8,
                           kind="ExternalInput")
    w2_e = nc.dram_tensor("w2", [NHID, OUTD], bf16, kind="ExternalInput")
    b1p_e = nc.dram_tensor("b1p", [P, JB], f32, kind="ExternalInput")
    bnsc_e = nc.dram_tensor("bnsc", [P, OB], f32, kind="ExternalInput")
    bnbi_e = nc.dram_tensor("bnbi", [P, OB], f32, kind="ExternalInput")
    uco_e = nc.dram_tensor("uco", [1, NHID], bf16, kind="ExternalInput")
    wco_e = nc.dram_tensor("wco", [1, ROWS], bf16, kind="ExternalInput")
    # outT: [OUTD, ROWS]; the host transposes each core's block.
    out_e = nc.dram_tensor("out", [OUTD, ROWS], f32, kind="ExternalOutput")

    groups = [list(range(NCORES))]

    def allgather(g_in, g_out):
        nc.gpsimd.collective_compute(
            "AllGather", mybir.AluOpType.bypass, replica_groups=groups,
            ins=[g_in[:]], outs=[g_out[:]])

    with tile.TileContext(nc) as tc:
        with (
            tc.tile_pool(name="const", bufs=1) as const,
            tc.tile_pool(name="sloc", bufs=1) as sloc_p,
            tc.tile_pool(name="h1", bufs=1) as h1_p,
            tc.tile_pool(name="zsb", bufs=1) as z_p,
            tc.tile_pool(name="schunk", bufs=13) as schunk_p,
            tc.tile_pool(name="zchunk", bufs=12) as zchunk_p,
            tc.tile_pool(name="astream", bufs=10) as astream,
            tc.tile_pool(name="afull", bufs=32) as afull_p,
            tc.tile_pool(name="outsb", bufs=1) as outsb_p,
            tc.tile_pool(name="dram", bufs=1, space="DRAM") as dram,
        ):
            # ---- constants into SBUF (w1/x first: phase 1 needs them;
            # x in halves so S-chunk-0 production waits on only 1 MB)
            w1_sb = const.tile([P, CP, 2, NHID], f8)
            nc.sync.dma_start(
                w1_sb[:],
                w1p_e[:].rearrange("(cp p) (t j) -> p cp t j", p=P, t=2))
            x_sb = []
            for h in range(2):
                xh = const.tile([P, CP, 2, ROWS // 2], f8)
                nc.sync.dma_start(
                    xh[:],
                    xp_e[h * CP * P:(h + 1) * CP * P, :].rearrange(
                        "(cp p) (t i) -> p cp t i", p=P, t=2))
                x_sb.append(xh)
            b1p_sb = const.tile([P, JB], f32)
            nc.sync.dma_start(b1p_sb[:], b1p_e[:])
            bnsc_sb = const.tile([P, OB], f32)
            nc.sync.dma_start(bnsc_sb[:], bnsc_e[:])
            bnbi_sb = const.tile([P, OB], f32)
            nc.sync.dma_start(bnbi_sb[:], bnbi_e[:])
            uco_sb = const.tile([1, NHID], bf16)
            nc.sync.dma_start(uco_sb[:], uco_e[:])
            wco_sb = const.tile([1, ROWS], bf16)
            nc.sync.dma_start(wco_sb[:], wco_e[:])
            xt_sb = const.tile([P, CP, 2, TB * P], f8)
            nc.sync.dma_start(
                xt_sb[:],
                xtp_e[:].rearrange("(cp p) (t i) -> p cp t i", p=P, t=2))
            w2_sb = const.tile([P, JB, OUTD], bf16)
            nc.sync.dma_start(
                w2_sb[:], w2_e[:].rearrange("(jb p) o -> p jb o", p=P))

            # ---- DRAM bounce buffers for the chunked fp8 collectives
            s_bounce = [dram.tile([2 * P, 2 * NHID], f8, name=f"sb{c}")
                        for c in range(GC)]
            s_all = [dram.tile([2 * P * NCORES, 2 * NHID], f8,
                               addr_space="Shared", name=f"sa{c}")
                     for c in range(GC)]
            z_bounce = [dram.tile([2 * P, 2 * OUTD], f8, name=f"zb{c}")
                        for c in range(GC)]
            z_all = [dram.tile([2 * P * NCORES, 2 * OUTD], f8,
                               addr_space="Shared", name=f"za{c}")
                     for c in range(GC)]

            def s_evict(dst, ps, ib):
                # alternate scalar/vector so back-to-back evictions overlap
                if ib % 2 == 0:
                    nc.scalar.activation(dst, ps[:], AF.Copy, scale=0.125)
                else:
                    nc.vector.tensor_scalar(dst, ps[:], 0.125, 0.0, MUL, ADD)

            # ---- phase 1: S_k = x[R_k] @ W1 in fp8 DR; bounce+gather/chunk
            s_loc = sloc_p.tile([P, IB, NHID], f8)
            with tc.tile_pool(name="ps1", bufs=2, space="PSUM") as ps1:
                # PE warmup: keep the tensor clock ramping while the
                # phase-1 operands stream in (output never read)
                dummy = const.tile([P, 2, HF], f8)
                nc.vector.memset(dummy[:], 0)
                ps_w = ps1.tile([P, HF], f32, tag="warm")
                for _ in range(NWARM):
                    nc.tensor.matmul(ps_w[:], dummy[:, :, 0:P], dummy[:],
                                     start=True, stop=True, perf_mode=DR)
                for c in range(GC):
                    for t in range(QT):
                        ib = c * QT + t
                        ps = ps1.tile([P, NHID], f32, tag="s")
                        for cp in range(CP):
                            nc.tensor.matmul(
                                ps[:],
                                x_sb[ib // QT][:, cp, :,
                                               (ib % QT) * P:
                                               (ib % QT + 1) * P],
                                w1_sb[:, cp, :, :],
                                start=(cp == 0), stop=(cp == CP - 1),
                                perf_mode=DR,
                            )
                        s_evict(s_loc[:, ib, :], ps, ib)
                    nc.sync.dma_start(
                        s_bounce[c][:].rearrange(
                            "(qq p) (t j) -> p qq t j", p=P, t=2),
                        s_loc[:, c * QT:(c + 1) * QT, :].rearrange(
                            "p (qq t) j -> p qq t j", qq=2))
                    allgather(s_bounce[c], s_all[c])
                # tail: redundantly compute S rows of the TAIL (c,k) groups
                # locally (identical on every core) so phase 2 has work
                # while the first gather is still in flight
                s_tail = sloc_p.tile([P, TB, NHID], f8)
                for tb in range(TB):
                    ps = ps1.tile([P, NHID], f32, tag="s")
                    for cp in range(CP):
                        nc.tensor.matmul(
                            ps[:],
                            xt_sb[:, cp, :, tb * P:(tb + 1) * P],
                            w1_sb[:, cp, :, :],
                            start=(cp == 0), stop=(cp == CP - 1),
                            perf_mode=DR,
                        )
                    s_evict(s_tail[:, tb, :], ps, tb)

            # gathered-S staging: chunk c, core-block k -> 2 m-tile pairs
            s_sb = [[None] * NCORES for _ in range(GC)]

            def stage_s(c, k):
                tile_ = schunk_p.tile([P, 2, 2, NHID], f8, tag="schunk")
                nc.sync.dma_start(
                    tile_[:],
                    s_all[c][k * 2 * P:(k + 1) * 2 * P, :]
                    .rearrange("(qq p) (t j) -> p qq t j", p=P, t=2))
                s_sb[c][k] = tile_

            h1T = h1_p.tile([P, JB, ROWS], bf16)
            z_sb = z_p.tile([P, IB, OUTD], f8)

            # phase-4 adj stream, prefetched during phase 2 (1 pair-tile
            # per phase-2 group keeps DMA below the HBM ceiling)
            a4_sb = {}
            p4_seq = [(c, k) for c in range(GC) for k in range(NCORES)]
            a4_queue = [(c, k, qq) for c, k in p4_seq for qq in range(2)]

            def load_a4():
                # scalar-engine DGE queue: never blocked by gather-gated
                # staging descriptors on the sync queue
                if a4_queue:
                    c, k, qq = a4_queue.pop(0)
                    g = 4 * k + 2 * c + qq
                    t = afull_p.tile([P, 2, ROWS], f8, tag="afull")
                    nc.scalar.dma_start(
                        t[:],
                        adjp_e[g * P:(g + 1) * P, :]
                        .rearrange("p (t i) -> p t i", t=2))
                    a4_sb[(c, k, qq)] = t

            # traversal: tail groups first (local S), then chunk-major
            order = TAIL + [(0, k) for k in range(NCORES)] \
                + [(1, k) for k in range(NCORES) if (1, k) not in TAIL]
            staged_seq = [ck for ck in order if ck not in TAIL]

            def a_load(ih, g):
                # scalar-engine DGE queue, same reasoning as load_a4
                t = astream.tile([P, 2, HF], f8, tag="ahalf")
                nc.scalar.dma_start(
                    t[:],
                    ifp_e[(ih * (MT // 2) + g) * P:
                          (ih * (MT // 2) + g + 1) * P, :]
                    .rearrange("p (t f) -> p t f", t=2))
                return t

            # ---- phase 2+3, i-half pass ih: accumulate h1T half in fp8
            # DR, add the rank-1 correction, emit z half (bf16), and fire
            # the Z allgather chunk for that half mid-kernel.
            def l1_pass(ih, ps2, ps3, pre_a, nxt_a):
                psum_h = [ps2.tile([P, HF], f32, name=f"ph{jb}_{ih}",
                                   tag=f"ph{jb}")
                          for jb in range(JB)]
                n_staged = 0
                n = 0
                for gi, (c, k) in enumerate(order):
                    tail_i = TAIL.index((c, k)) if (c, k) in TAIL else -1
                    for qq in range(2):
                        g = 4 * k + 2 * c + qq
                        key = (ih, g)
                        a_tile = pre_a.pop(key, None)
                        if a_tile is None:
                            a_tile = a_load(ih, g)
                        if tail_i >= 0:
                            s_src = s_tail[:, 4 * tail_i + 2 * qq:
                                           4 * tail_i + 2 * qq + 2, :]
                        else:
                            s_src = s_sb[c][k][:, qq, :, :]
                        for jb in range(JB):
                            nc.tensor.matmul(
                                psum_h[jb][:],
                                s_src[:, :, jb * P:(jb + 1) * P],
                                a_tile[:],
                                start=(n == 0), stop=False,
                                perf_mode=DR,
                            )
                        n += 1
                    # gather-dependent staging strictly AFTER this group's
                    # independent loads (single in-order DMA queue)
                    if ih == 0 and n_staged < len(staged_seq):
                        stage_s(*staged_seq[n_staged])
                        n_staged += 1
                    load_a4()
                # preload the next pass's first a-tiles before the
                # epilogue chain so its start is not DMA-gated
                for c, k in order[:2]:
                    for qq in range(2):
                        g = 4 * k + 2 * c + qq
                        if nxt_a is not None:
                            nxt_a[(ih + 1, g)] = a_load(ih + 1, g)
                # rank-1 correction closes each accumulation group
                for jb in range(JB):
                    nc.tensor.matmul(
                        psum_h[jb][:],
                        uco_sb[:, jb * P:(jb + 1) * P],
                        wco_sb[:, ih * HF:(ih + 1) * HF],
                        start=False, stop=True,
                    )
                # epilogue: relu+bias into h1T half (scalar/vector split)
                for jb in range(JB):
                    dst = h1T[:, jb, ih * HF:(ih + 1) * HF]
                    if jb % 2 == 0:
                        nc.scalar.activation(dst, psum_h[jb][:], AF.Relu,
                                             bias=b1p_sb[:, jb:jb + 1])
                    else:
                        nc.vector.tensor_scalar(dst, psum_h[jb][:],
                                                b1p_sb[:, jb:jb + 1], 0.0,
                                                ADD, MAX)
                # z for this half's i-blocks (bf16), bounce, gather chunk
                for t in range(QT):
                    ib = ih * QT + t
                    ps = ps3.tile([P, OUTD], f32, tag="z")
                    for jb in range(JB):
                        nc.tensor.matmul(
                            ps[:],
                            h1T[:, jb, ib * P:(ib + 1) * P],
                            w2_sb[:, jb, :],
                            start=(jb == 0), stop=(jb == JB - 1),
                        )
                    if t % 2 == 0:
                        nc.scalar.activation(z_sb[:, ib, :], ps[:], AF.Copy)
                    else:
                        nc.vector.tensor_scalar(z_sb[:, ib, :], ps[:],
                                                1.0, 0.0, MUL, ADD)
                nc.sync.dma_start(
                    z_bounce[ih][:].rearrange(
                        "(qq p) (t o) -> p qq t o", p=P, t=2),
                    z_sb[:, ih * QT:(ih + 1) * QT, :].rearrange(
                        "p (qq t) o -> p qq t o", qq=2))
                allgather(z_bounce[ih], z_all[ih])

            with (
                tc.tile_pool(name="ps2", bufs=1, space="PSUM") as ps2,
                tc.tile_pool(name="ps3", bufs=2, space="PSUM") as ps3,
            ):
                handoff = {}
                l1_pass(0, ps2, ps3, {}, handoff)
                l1_pass(1, ps2, ps3, handoff, None)

            # ---- phase 4: outT[o, i] = sum_m Z[m, o] * adjT[m, i] in fp8
            # DR, BN fused on the PSUM evict.  Chunk-major over Z chunks.
            outT_sb = outsb_p.tile([P, OB, ROWS], f32)
            zc_sb = [[None] * NCORES for _ in range(GC)]

            def stage_z(c, k):
                tile_ = zchunk_p.tile([P, 2, 2, OUTD], f8, tag="zchunk")
                nc.sync.dma_start(
                    tile_[:],
                    z_all[c][k * 2 * P:(k + 1) * 2 * P, :]
                    .rearrange("(qq p) (t o) -> p qq t o", p=P, t=2))
                zc_sb[c][k] = tile_

            with tc.tile_pool(name="ps4", bufs=1, space="PSUM") as ps4:
                psum_o = [[ps4.tile([P, HF], f32, name=f"po{ob}_{ih}",
                                    tag=f"po{ob}_{ih}")
                           for ih in range(IH)] for ob in range(OB)]
                # drain any adj tiles not yet prefetched, then stage all
                # z chunks (c=0 is already gathered; c=1 staging may wait
                # on the second Z gather and blocks only the output DMA)
                while a4_queue:
                    load_a4()
                for k in range(NCORES):
                    stage_z(0, k)
                for k in range(NCORES):
                    stage_z(1, k)
                first = True
                for c, k in p4_seq:
                    zc = zc_sb[c][k]
                    final_grp = (c == GC - 1 and k == NCORES - 1)
                    if not final_grp:
                        for qq in range(2):
                            a_tile = a4_sb[(c, k, qq)]
                            for ob in range(OB):
                                for ihh in range(IH):
                                    nc.tensor.matmul(
                                        psum_o[ob][ihh][:],
                                        zc[:, qq, :, ob * P:(ob + 1) * P],
                                        a_tile[:, :,
                                               ihh * HF:(ihh + 1) * HF],
                                        start=first, stop=False,
                                        perf_mode=DR,
                                    )
                            first = False
                    else:
                        # last group: finish ob=0's accumulators first so
                        # their eviction overlaps ob=1's final matmuls
                        for ob in range(OB):
                            for qq in range(2):
                                a_tile = a4_sb[(c, k, qq)]
                                for ihh in range(IH):
                                    nc.tensor.matmul(
                                        psum_o[ob][ihh][:],
                                        zc[:, qq, :, ob * P:(ob + 1) * P],
                                        a_tile[:, :,
                                               ihh * HF:(ihh + 1) * HF],
                                        start=False, stop=(qq == 1),
                                        perf_mode=DR,
                                    )
                # fused BN affine on PSUM evict: out = psum*scale + bias
                for ob in range(OB):
                    for ihh in range(IH):
                        nc.vector.tensor_scalar(
                            outT_sb[:, ob, ihh * HF:(ihh + 1) * HF],
                            psum_o[ob][ihh][:],
                            bnsc_sb[:, ob:ob + 1],
                            bnbi_sb[:, ob:ob + 1],
                            MUL, ADD)
                    nc.sync.dma_start(
                        out_e[ob * P:(ob + 1) * P, :], outT_sb[:, ob, :])

    nc.compile()
    return nc


def _get_nc():
    if "nc" not in _cache:
        _cache["nc"] = _build()
    return _cache["nc"]


def _pack_pairs(mat_kx, width):
    """[K, width] -> packed [K/256*128, 2*width]: row g*128+p holds the
    DoubleRow pair's two k-tiles (rows 2g*128+p and (2g+1)*128+p)."""
    k = mat_kx.shape[0]
    return np.ascontiguousarray(
        mat_kx.reshape(k // 256, 2, P, width)
        .transpose(0, 2, 1, 3).reshape(k // 2, 2 * width))


def kernel(x, IFadj, adj, W1, b1, W2, b2, bn_gamma, bn_beta, bn_mean, bn_var):
    from concourse.bass_utils import run_bass_kernel_spmd

    x = np.asarray(x, dtype=np.float32)
    IFadj = np.asarray(IFadj, dtype=np.float32)
    adj = np.asarray(adj, dtype=np.float32)
    W1 = np.asarray(W1, dtype=np.float32)
    b1 = np.asarray(b1, dtype=np.float32)
    W2 = np.asarray(W2, dtype=np.float32)
    b2 = np.asarray(b2, dtype=np.float32)
    bn_gamma = np.asarray(bn_gamma, dtype=np.float32)
    bn_beta = np.asarray(bn_beta, dtype=np.float32)
    bn_mean = np.asarray(bn_mean, dtype=np.float32)
    bn_var = np.asarray(bn_var, dtype=np.float32)

    # host-side prep: fp8 casts, DoubleRow pair packing, correction vecs
    x8 = x.astype(_F8)
    W18 = (8.0 * W1).astype(_F8)
    w1p = _pack_pairs(W18, NHID)
    w2b = W2.astype(_BF16)
    b1p = np.ascontiguousarray(b1.reshape(JB, P).T)  # [P, JB]
    inv = bn_gamma / np.sqrt(bn_var + BN_EPS)
    bias_tot = b2 * inv + bn_beta - bn_mean * inv
    bnsc = np.ascontiguousarray(inv.reshape(OB, P).T)       # [P, OB]
    bnbi = np.ascontiguousarray(bias_tot.reshape(OB, P).T)  # [P, OB]

    # rank-1 correction: u = colsum(S_true) - colsum(S8_device-replica)
    S_host = (x8.astype(np.float32) @ W18.astype(np.float32)) * 0.125
    S8 = S_host.astype(_F8)
    u = (x @ W1).sum(0) - S8.astype(np.float32).sum(0)
    uco = np.ascontiguousarray(u.astype(_BF16).reshape(1, NHID))

    IFadj8 = IFadj.astype(_F8)
    adj8 = adj.astype(_F8)
    abar = IFadj8.astype(np.float32).sum(1) / float(N)  # [N]

    # x rows for the TAIL m-tiles (c,k) in TAIL -> mt = 8k+4c+q
    tail_rows = np.concatenate(
        [x8[(8 * k + 4 * c) * P:(8 * k + 4 * c + QT) * P]
         for c, k in TAIL])
    xtp = _pack_pairs(np.ascontiguousarray(tail_rows.T), TB * P)

    in_maps = []
    for k in range(NCORES):
        r0, r1 = k * ROWS, (k + 1) * ROWS
        ifT = np.ascontiguousarray(IFadj8[r0:r1].T)  # [N, ROWS]
        # per-ih-half pair packing: row (ih*4096 + g*128 + p)
        ifp = np.ascontiguousarray(
            ifT.reshape(MT // 2, 2, P, IH, HF)
            .transpose(3, 0, 2, 1, 4).reshape(IH * (MT // 2) * P, 2 * HF))
        adjp = _pack_pairs(np.ascontiguousarray(adj8[r0:r1].T), ROWS)
        # x packed per i-half: rows [h*512 + cp*128 + p], cols (t, i')
        xp = np.concatenate(
            [_pack_pairs(np.ascontiguousarray(
                x8[r0 + h * (ROWS // 2):r0 + (h + 1) * (ROWS // 2)].T),
                ROWS // 2)
             for h in range(2)], axis=0)
        wco = np.ascontiguousarray(abar[r0:r1].astype(_BF16).reshape(1, ROWS))
        in_maps.append({
            "xp": xp,
            "xtp": xtp,
            "ifp": ifp,
            "adjp": adjp,
            "w1p": w1p,
            "w2": w2b,
            "b1p": b1p,
            "bnsc": bnsc,
            "bnbi": bnbi,
            "uco": uco,
            "wco": wco,
        })

    global _last_in_maps
    _last_in_maps = in_maps

    nc = _get_nc()
    try:
        res = run_bass_kernel_spmd(nc, in_maps, list(range(NCORES)))
    except Exception:
        # transient device wedge (NRT_EXEC_UNIT_UNRECOVERABLE etc.) --
        # a straight retry has been observed to recover
        import time
        time.sleep(2.0)
        res = run_bass_kernel_spmd(nc, in_maps, list(range(NCORES)))
    # per-core output is outT [OUTD, ROWS]; transpose back and stack rows
    return np.concatenate(
        [np.ascontiguousarray(res.results[k]["out"].T)
         for k in range(NCORES)], axis=0)
